# revision 67
# baseline (speedup 1.0000x reference)
"""Trainium2 Bass kernel: Aaren-style online-softmax linear-attention scan.

Math (per (b,h) pair, per timestep t):
    alpha_t = q_t . k_t                       (scalar)
    y_t = sum_{s<=t} exp(alpha_s - C_t) v_s / sum_{s<=t} exp(alpha_s - C_t)
for any stabilizer C_t >= running max (the ratio is invariant). We use the
running *chunk* max M_c, mathematically identical to the reference.

Layout: host pre-permutes each (b,h) pair's [N, D] into
    x_perm[p, c, d] = x[c*128 + p, d]       (fp16 on the wire)
i.e. [128 partitions = in-chunk time, 32 chunks x 128 features]. N = 4096 =
32*128 exactly: no padding. q|k|v are shipped as ONE consolidated dram
tensor per pair (one qk + one v DMA), halving HBM bytes vs fp32 and keeping
the single DMA_ENGINES resource saturated.

Per pair (software-pipelined as phase1a(p) | chunks(p-1) | phase1b(p) so no
engine queue ever waits on a neighbour pair's dependencies):
  phase1a: prod = q*k (DVE fp16 2x), A[p,c] = reduce_d (DVE, fp32 out),
    chunk maxes via PE transpose + DVE max-reduce, running max via a 1-row
    max-scan, W = exp(A - M) (Act, fp32), gamma logits.
  phase1b: wv = v * W[:,c] (Pool per chunk, bf16 out -- bf16 keeps fp32's
    exponent range; fp16 wv underflows when a chunk's max lands late in the
    chunk). Carries WITHOUT a serial chain: per-chunk sums S_c[d] via tiny
    one-column matmuls (stationary = wv chunk), ONE 128-partition
    tensor_tensor_scan C_c = gamma_c*C_{c-1} + S_c, then carry rows
    gamma_c*C_{c-1} formed as gb * shift(C) (direct product -- a C - S
    subtraction cancels catastrophically) and transposed once to
    crows [32, 128] bf16. Denominator: cumw = U2 @ W plus rank-1
    ones (x) gamma*shift(D) from a 1-row mult/add scan; R = 1/d.
  chunks: per 4-chunk group, ONE 512-wide matmul
    psum[t,(c,d)] = sum_s U2[s,t] wv[s,(c,d)] plus four per-chunk rank-1
    carry adds (stationary = SEL[:, c*128:(c+1)*128] with
    SEL[s,c*128+t] = (s==c), moving = crows -- keeps every matmul operand at
    base partition 0). Evacs alternate: odd groups as ONE batched DVE
    tensor_tensor (psum * R broadcast along d), even groups as four Act
    activations with per-chunk scale R[:,c]. y DMA triggers ride the Act
    queue so they never head-of-line-block input triggers on SP's FIFO.

The last pair's scales are split Pool/DVE and its y DMA is issued in
quarters to shorten the pipeline ramp-down.

Sharding: B*H = 64 pairs -> 8 pairs per NeuronCore, no cross-core traffic.
fp16 wire halves HBM traffic; all accumulation stays fp32 (PSUM / DVE).
TimelineSim makespan: 117,667 ns (baseline 355,669 ns).
"""

import sys

for _p in ("/root/.axon_site/_ro/trn_rl_repo", "/opt/trn_rl_repo"):
    if _p not in sys.path:
        sys.path.append(_p)

import numpy as np

B, H, N, D = 4, 16, 4096, 128
NCORES = 8
PAIRS = B * H // NCORES  # 8 (b,h) pairs per core

CH = 128           # timesteps per chunk
NCH = N // CH      # 32 chunks
FW = NCH * D       # free width of the packed per-pair tiles (4096)
GW = 4 * D         # chunk-group width: 4 chunks per PSUM bank
NG = NCH // 4      # 8 chunk groups
NEG = -3.0e38


def build_nc(pairs=PAIRS, n=N, mode="full"):
    import concourse.tile as tile
    from concourse import bacc, mybir
    from concourse.bass import broadcast_tensor_aps
    from contextlib import ExitStack

    do_dma = mode in ("full", "dma")
    do_cmp = mode in ("full", "compute")

    f16 = mybir.dt.float16
    bf16 = mybir.dt.bfloat16
    f32 = mybir.dt.float32
    Alu = mybir.AluOpType
    Act = mybir.ActivationFunctionType
    X = mybir.AxisListType.X

    nch = n // CH
    fw = nch * D
    ng = nch // 4

    nc = bacc.Bacc("TRN2", target_bir_lowering=False, debug=False)

    qkvd = nc.dram_tensor("qkv", [pairs, 128, 3 * fw], f16,
                          kind="ExternalInput")
    yd = nc.dram_tensor("y", [pairs, 128, fw], f16, kind="ExternalOutput")

    with tile.TileContext(nc) as tc, ExitStack() as ctx:
        cpool = ctx.enter_context(tc.tile_pool(name="consts", bufs=1))
        qkpool = ctx.enter_context(tc.tile_pool(name="qkv", bufs=5))
        prpool = ctx.enter_context(tc.tile_pool(name="prod", bufs=2))
        wvpool = ctx.enter_context(tc.tile_pool(name="wv", bufs=4))
        ypool = ctx.enter_context(tc.tile_pool(name="yy", bufs=2))
        smpool = ctx.enter_context(tc.tile_pool(name="sm", bufs=3))
        scpool = ctx.enter_context(
            tc.tile_pool(name="scr", bufs=3, space="PSUM"))
        pspool = ctx.enter_context(
            tc.tile_pool(name="ps", bufs=5, space="PSUM"))

        # ---- constants -------------------------------------------------
        iota_f = cpool.tile([128, 128], f32, tag="iotaf")
        nc.gpsimd.iota(iota_f[:], [[1, 128]], channel_multiplier=0,
                       allow_small_or_imprecise_dtypes=True)
        iota_p = cpool.tile([128, 1], f32, tag="iotap")
        nc.gpsimd.iota(iota_p[:], [[0, 1]], channel_multiplier=1,
                       allow_small_or_imprecise_dtypes=True)
        # u2[s, t] = 1.0 if t >= s else 0.0 (full lower-triangular)
        u2 = cpool.tile([128, 128], bf16, tag="u2")
        nc.vector.tensor_scalar(u2[:], iota_f[:], iota_p[:], None, Alu.is_ge)
        u2_32 = cpool.tile([128, 128], f32, tag="u2f32")
        nc.vector.tensor_scalar(u2_32[:], iota_f[:], iota_p[:], None,
                                Alu.is_ge)
        ident = cpool.tile([128, 128], f32, tag="ident")
        nc.vector.tensor_scalar(ident[:], iota_f[:], iota_p[:], None,
                                Alu.is_equal)
        ones_row32 = cpool.tile([1, 128], f32, tag="onesrow32")
        nc.gpsimd.memset(ones_row32[:], 1.0)
        ones_col32 = cpool.tile([128, 1], f32, tag="onescol32")
        nc.gpsimd.memset(ones_col32[:], 1.0)
        ones_col = cpool.tile([128, 1], bf16, tag="onescol")
        nc.gpsimd.memset(ones_col[:], 1.0)
        # SEL[s, c*128 + t] = 1.0 if s == c else 0: selector stationary used
        # to broadcast carry row c of crows to every output partition.
        # jrep is startup-only scratch; it borrows a wv pool slot.
        jrep = wvpool.tile([32, nch * 128], bf16, tag="wv", name="jrep")
        nc.gpsimd.iota(jrep[:], [[1, nch], [0, 128]], channel_multiplier=0,
                       allow_small_or_imprecise_dtypes=True)
        iota_p32 = cpool.tile([32, 1], f32, tag="iotap32")
        nc.gpsimd.iota(iota_p32[:], [[0, 1]], channel_multiplier=1,
                       allow_small_or_imprecise_dtypes=True)
        sel = cpool.tile([32, nch * 128], bf16, tag="sel")
        nc.gpsimd.tensor_scalar(sel[:], jrep[:], iota_p32[:], None,
                                Alu.is_equal)

        qt, kt, vt, yt, wvt = {}, {}, {}, {}, {}
        Wt, Rt, gmt, crt = {}, {}, {}, {}
        scrt = {}

        def load(p):
            qkv = qkpool.tile([128, 3 * fw], f16, tag="qkv", name=f"qkv{p}")
            qt[p] = qkv[:, 0:fw]
            kt[p] = qkv[:, fw:2 * fw]
            vt[p] = qkv[:, 2 * fw:3 * fw]
            if do_dma:
                nc.sync.dma_start(qkv[:, 0:2 * fw], qkvd[p][:, 0:2 * fw])
                nc.sync.dma_start(qkv[:, 2 * fw:3 * fw],
                                  qkvd[p][:, 2 * fw:3 * fw])

        if not do_cmp:
            for p in range(pairs):
                load(p)
                if do_dma:
                    nc.sync.dma_start(yd[p], vt[p])
            nc.compile()
            return nc

        def phase1a(p):
            """Input DMA + alpha, chunk/running maxes, A-M, exps."""
            load(p)
            prod = prpool.tile([128, fw], f16, tag="pr", name=f"pr{p}")
            nc.vector.tensor_mul(prod[:], qt[p], kt[p])
            A = smpool.tile([128, nch], f32, tag="A", name=f"A{p}")
            nc.vector.tensor_reduce(
                A[:], prod[:].rearrange("p (c d) -> p c d", d=D),
                axis=X, op=Alu.add)

            scr = scpool.tile([128, 512], f32, tag="scr", name=f"scr{p}")
            scrt[p] = scr
            nc.tensor.transpose(scr[0:nch, 0:128], A[:], ident[:])
            mu = smpool.tile([128, 1], f32, tag="mu")
            nc.vector.tensor_reduce(mu[0:nch, :], scr[0:nch, 0:128],
                                    axis=X, op=Alu.max)
            nc.tensor.transpose(scr[0:1, 128:128 + nch], mu[0:nch, :],
                                ident[0:nch, 0:nch])
            mrow = smpool.tile([1, nch], f32, tag="mrow")
            nc.vector.tensor_copy(mrow[0:1, :], scr[0:1, 128:128 + nch])
            Mrow = smpool.tile([1, nch], f32, tag="Mrow")
            nc.vector.tensor_tensor_scan(Mrow[0:1, :], mrow[0:1, :],
                                         mrow[0:1, :], initial=NEG,
                                         op0=Alu.max, op1=Alu.max)
            # gamma logits: g2 = M_{c-1} - M_c (g2_0 = 0)
            g1 = smpool.tile([1, nch], f32, tag="g1")
            nc.vector.tensor_copy(g1[0:1, 1:nch], Mrow[0:1, 0:nch - 1])
            nc.vector.tensor_copy(g1[0:1, 0:1], Mrow[0:1, 0:1])
            g2 = smpool.tile([1, nch], f32, tag="g2", name=f"g2_{p}")
            nc.vector.tensor_sub(g2[0:1, :], g1[0:1, :], Mrow[0:1, :])
            # A - M broadcast
            nc.tensor.matmul(scr[0:128, 160:160 + nch], ones_row32[0:1, :],
                             Mrow[0:1, :], start=True, stop=True)
            AmM = smpool.tile([128, nch], f32, tag="AmM", name=f"AmM{p}")
            nc.vector.tensor_sub(AmM[:], A[:], scr[0:128, 160:160 + nch])
            W = smpool.tile([128, nch], f32, tag="W", name=f"W{p}")
            Wt[p] = W
            nc.scalar.activation(W[:], AmM[:], Act.Exp)
            gm = smpool.tile([1, nch], f32, tag="gm", name=f"gm{p}")
            gmt[p] = gm
            nc.scalar.activation(gm[0:1, :], g2[0:1, :], Act.Exp)

        def phase1b(p):
            """v scaling, denominators, carry chain, crowsX."""
            scr = scrt[p]
            W = Wt[p]
            gm = gmt[p]

            # scale v rows: wv = v * W[:, c] (Pool), freeing the qkv tile
            wv = wvpool.tile([128, fw], bf16, tag="wv", name=f"wv{p}")
            wvt[p] = wv
            for c in range(nch):
                cs = c * D
                if p == pairs - 1 and c % 2 == 1:
                    eng = nc.vector
                elif p == pairs - 2 and c % 4 == 1:
                    eng = nc.vector
                else:
                    eng = nc.gpsimd
                eng.tensor_scalar_mul(wv[:, cs:cs + D],
                                      vt[p][:, cs:cs + D],
                                      W[:, c:c + 1])

            # denominator
            nc.tensor.matmul(scr[0:1, 200:200 + nch], ones_col32[:], W[:],
                             start=True, stop=True)
            swrow = smpool.tile([1, nch], f32, tag="swrow")
            nc.vector.tensor_copy(swrow[0:1, :], scr[0:1, 200:200 + nch])
            Drow = smpool.tile([1, nch], f32, tag="Drow")
            nc.vector.tensor_tensor_scan(Drow[0:1, :], gm[0:1, :],
                                         swrow[0:1, :], initial=0.0,
                                         op0=Alu.mult, op1=Alu.add)
            Dsh = smpool.tile([1, nch], f32, tag="Dsh")
            nc.vector.memset(Dsh[0:1, 0:1], 0.0)
            nc.vector.tensor_copy(Dsh[0:1, 1:nch], Drow[0:1, 0:nch - 1])
            adj = smpool.tile([1, nch], f32, tag="adj")
            nc.vector.tensor_mul(adj[0:1, :], gm[0:1, :], Dsh[0:1, :])
            dps = scr[0:128, 224:224 + nch]
            nc.tensor.matmul(dps, u2_32[:], W[:], start=True, stop=False)
            nc.tensor.matmul(dps, ones_row32[0:1, :], adj[0:1, :],
                             start=False, stop=True)
            R = smpool.tile([128, nch], f32, tag="R", name=f"R{p}")
            Rt[p] = R
            nc.vector.reciprocal(R[:], dps)

            # gamma broadcast for the 128-lane scan
            nc.tensor.matmul(scr[0:128, 256:256 + nch], ones_row32[0:1, :],
                             gm[0:1, :], start=True, stop=True)
            gb = smpool.tile([128, nch], f32, tag="gb")
            nc.vector.tensor_copy(gb[:], scr[0:128, 256:256 + nch])

            # numerator carries: per-chunk sums -> scan -> carry rows
            ST = scr[0:128, 288:288 + nch]
            for c in range(nch):
                nc.tensor.matmul(ST[:, c:c + 1], wv[:, c * D:(c + 1) * D],
                                 ones_col[:], start=True, stop=True)
            C = smpool.tile([128, nch], f32, tag="C")
            nc.vector.tensor_tensor_scan(C[:], gb[:], ST, initial=0.0,
                                         op0=Alu.mult, op1=Alu.add)
            Csh = smpool.tile([128, nch], f32, tag="Csh")
            nc.vector.memset(Csh[:, 0:1], 0.0)
            nc.vector.tensor_copy(Csh[:, 1:nch], C[:, 0:nch - 1])
            CmS = smpool.tile([128, nch], f32, tag="CmS")
            nc.vector.tensor_mul(CmS[:], gb[:], Csh[:])
            nc.tensor.transpose(scr[0:nch, 320:320 + 128], CmS[:], ident[:])
            crows = smpool.tile([nch, 128], bf16, tag="cr", name=f"cr{p}")
            crt[p] = crows
            nc.vector.tensor_copy(crows[:], scr[0:nch, 320:320 + 128])


        def chunks(p):
            """Batched prefix matmuls + per-chunk rank-1 carries + evacs."""
            wv, crows, R = wvt[p][:], crt[p], Rt[p]
            yt[p] = ypool.tile([128, fw], f16, tag="ya", name=f"ya{p}")
            for g in range(ng):
                gs = g * GW
                ps = pspool.tile([128, 512], f32, tag="cps",
                                 name=f"cps{p}_{g}")
                nc.tensor.matmul(ps[:, :], u2[:], wv[:, gs:gs + GW],
                                 start=True, stop=False)
                for j in range(4):
                    c = 4 * g + j
                    nc.tensor.matmul(ps[:, j * D:(j + 1) * D],
                                     sel[:, c * D:(c + 1) * D], crows[:, :],
                                     start=False, stop=(j == 3))
                if g % 2 == 1:
                    # batched DVE evac: y_g = ps * R (R broadcast along d)
                    y3 = yt[p][:, gs:gs + GW].rearrange(
                        "p (b d) -> p b d", d=D)
                    ps3 = ps[:, :].rearrange("p (b d) -> p b d", d=D)
                    r3 = R[:, 4 * g:4 * g + 4].rearrange(
                        "p (b o) -> p b o", o=1)
                    r3b, ps3b = broadcast_tensor_aps(r3, ps3)
                    nc.vector.tensor_tensor(y3, ps3b, r3b, op=Alu.mult)
                else:
                    for j in range(4):
                        c = 4 * g + j
                        nc.scalar.activation(yt[p][:, c * D:(c + 1) * D],
                                             ps[:, j * D:(j + 1) * D],
                                             Act.Copy, scale=R[:, c:c + 1])
            if do_dma:
                if p == pairs - 1:
                    qw = fw // 4
                    for qi in range(4):
                        nc.scalar.dma_start(yd[p][:, qi * qw:(qi + 1) * qw],
                                            yt[p][:, qi * qw:(qi + 1) * qw])
                else:
                    nc.scalar.dma_start(yd[p], yt[p][:])

        for p in range(pairs + 1):
            if p < pairs:
                phase1a(p)
            if p >= 1:
                chunks(p - 1)
            if p < pairs:
                phase1b(p)

    nc.compile()
    return nc


def pack_inputs(x, n=N):
    """[pairs_total, n, D] f32 -> [pairs_total, 128, nch*D] fp16 permuted."""
    nch = n // CH
    m = x.shape[0]
    xp = x.reshape(m, nch, CH, D).transpose(0, 2, 1, 3)  # [m, 128, nch, D]
    return np.ascontiguousarray(xp.reshape(m, 128, nch * D).astype(np.float16))


def unpack_output(yp, n=N):
    """[pairs_total, 128, nch*D] fp16 -> [pairs_total, n, D] f32."""
    nch = n // CH
    m = yp.shape[0]
    yv = yp.astype(np.float32).reshape(m, 128, nch, D)
    yv = yv.transpose(0, 2, 1, 3).reshape(m, nch * CH, D)
    return np.ascontiguousarray(yv)


_cached = {}


def _get_nc():
    if "nc" not in _cached:
        _cached["nc"] = build_nc()
    return _cached["nc"]


def run_on_hw(q, k, v, trace=False):
    """q,k,v: np [B,H,N,D] f32 -> (y [B,H,N,D], exec_time_ns or None)."""
    from concourse.bass_utils import run_bass_kernel_spmd

    nc = _get_nc()
    qp = pack_inputs(np.asarray(q, np.float32).reshape(B * H, N, D))
    kp = pack_inputs(np.asarray(k, np.float32).reshape(B * H, N, D))
    vp = pack_inputs(np.asarray(v, np.float32).reshape(B * H, N, D))
    qkvp = np.ascontiguousarray(np.concatenate([qp, kp, vp], axis=2))
    in_maps = [
        {"qkv": qkvp[c * PAIRS:(c + 1) * PAIRS]}
        for c in range(NCORES)
    ]
    try:
        res = run_bass_kernel_spmd(nc, in_maps, list(range(NCORES)), trace=trace)
    except Exception:
        if not trace:
            raise
        import traceback
        traceback.print_exc()
        print("trace=True path failed; retrying without trace", file=sys.stderr)
        res = run_bass_kernel_spmd(nc, in_maps, list(range(NCORES)), trace=False)
    yp = np.concatenate([np.asarray(res.results[c]["y"]) for c in range(NCORES)],
                        axis=0)
    return unpack_output(yp).reshape(B, H, N, D), res.exec_time_ns


def kernel(q, k, v):
    y, _ = run_on_hw(q, k, v, trace=False)
    return y
